# revision 23
# baseline (speedup 1.0000x reference)
"""Trainium2 Bass kernel for a DiT (DiffusionTransformer) block, 8-core SPMD.

Sharding:
  - LayerNorm1+modulate: token-sharded (512 tokens/core), AllGather of xn.
  - QKV + attention: head-sharded (2 heads x 2 batches per core), all tokens.
  - AllToAll re-shards attention output from head-major to token-major.
  - out_proj + LayerNorm2 + FFN: token-sharded, full weights streamed.

On-chip layout is feature-major ([feature partitions, token free]) everywhere;
matmuls run in float32r (full PE rate, ~tf32 precision); probs/V in bf16.
"""

import sys
from contextlib import ExitStack

sys.path.insert(0, "/opt/trn_rl_repo")

import numpy as np

import concourse.bass as bass
import concourse.tile as tile
from concourse import bacc, mybir
from concourse.bass_utils import run_bass_kernel_spmd

F32 = mybir.dt.float32
F32R = mybir.dt.float32r
BF16 = mybir.dt.bfloat16
AF = mybir.ActivationFunctionType
MUL = mybir.AluOpType.mult
ADD = mybir.AluOpType.add

NC_N = 8
B, S, D, H, HD, FF = 2, 2048, 1024, 16, 64, 4096
SL = S // NC_N          # 256 seq positions per core (token shard)
T = B * SL              # 512 tokens per core
NKT = D // 128          # 8 feature k-tiles
NFT = FF // 128         # 32 ffn feature tiles
EPS = 1e-5

_CACHE = {}


def _round_f32r(x, bits=10):
    x = np.ascontiguousarray(x, np.float32)
    u = x.view(np.uint32)
    shift = 23 - bits
    u = (u + np.uint32(1 << (shift - 1))) & np.uint32(~((1 << shift) - 1) & 0xFFFFFFFF)
    return u.view(np.float32)


def _build():
    nc = bacc.Bacc("TRN2", target_bir_lowering=False, debug=False, num_devices=NC_N)

    def inp(name, shape, dtype):
        return nc.dram_tensor(name, list(shape), dtype, kind="ExternalInput").ap()

    xfm = inp("xfm", [NKT, 128, B, SL], F32)
    xfmr = inp("xfmr", [NKT, 128, B, SL], F32R)
    wq_t = inp("wq_t", [NKT, 128, 128], F32R)
    wk_t = inp("wk_t", [NKT, 128, 128], F32R)
    wv_t = inp("wv_t", [NKT, 128, 128], F32R)
    bqkv = inp("bqkv", [128, 3], F32)
    wo_t = inp("wo_t", [NKT, NKT, 128, 128], F32R)
    wo_b = inp("wo_b", [128, NKT], F32)
    w1_t = inp("w1_t", [NFT, NKT, 128, 128], F32R)
    b1_t = inp("b1_t", [128, NFT], F32)
    w2_t = inp("w2_t", [NKT, NFT, 128, 128], F32R)
    b2_t = inp("b2_t", [128, NKT], F32)
    msh = inp("msh", [128, NKT, B, 4], F32)
    yout = nc.dram_tensor("yout", [NKT, 128, B, SL], F32, kind="ExternalOutput").ap()
    import os
    dbg = os.environ.get("KDBG") == "1"
    if dbg:
        x2_dump = nc.dram_tensor("x2_dump", [128, NKT, T], F32, kind="ExternalOutput").ap()
        xa_dump = nc.dram_tensor("xa_dump", [128, NKT, T], F32, kind="ExternalOutput").ap()
        send_dump = nc.dram_tensor("send_dump", [NC_N, 128, B, SL], F32R, kind="ExternalOutput").ap()
        q_dump = nc.dram_tensor("q_dump", [128, B, 4, 512], F32R, kind="ExternalOutput").ap()
        k_dump = nc.dram_tensor("k_dump", [128, B, 4, 512], F32R, kind="ExternalOutput").ap()
        v_dump = nc.dram_tensor("v_dump", [128, B, 16, 132], F32, kind="ExternalOutput").ap()
        po_dump = nc.dram_tensor("po_dump", [65, 512], F32, kind="ExternalOutput").ap()
        pt_dump2 = nc.dram_tensor("pt_dump2", [128, 2, 512], F32, kind="ExternalOutput").ap()

    with tile.TileContext(nc) as tc, ExitStack() as ctx:
        singles = ctx.enter_context(tc.tile_pool(name="singles", bufs=1))
        dram = ctx.enter_context(tc.tile_pool(name="dram", bufs=1, space="DRAM"))
        rows = ctx.enter_context(tc.tile_pool(name="rows", bufs=2))
        ps_stat = ctx.enter_context(tc.tile_pool(name="ps_stat", bufs=1, space="PSUM"))

        # ---------- resident constants ----------
        ones_f = singles.tile([128, 1], F32)
        nc.vector.memset(ones_f, 1.0)
        ones_r = singles.tile([128, 1], F32R)
        nc.vector.tensor_copy(out=ones_r, in_=ones_f)
        eps_sb = singles.tile([1, 1], F32)
        nc.vector.memset(eps_sb, EPS)
        msh_sb = singles.tile([128, NKT, B, 4], F32)
        nc.sync.dma_start(out=msh_sb, in_=msh)
        x2_sb = singles.tile([128, NKT, T], F32)
        xn2 = singles.tile([128, NKT, T], F32R)

        # ---------- helpers ----------
        def ln_stats_rows(ps_sum, ps_ss):
            """[1, T] psum sums -> (RSTD, C0) [128, T] sbuf f32 (broadcasted)."""
            mu = rows.tile([1, T], F32, tag="mu", name="mu")
            ex2 = rows.tile([1, T], F32, tag="ex2", name="ex2")
            nc.scalar.mul(mu, ps_sum, 1.0 / D)
            nc.scalar.mul(ex2, ps_ss, 1.0 / D)
            var = rows.tile([1, T], F32, tag="var", name="var")
            nc.vector.tensor_mul(var, mu, mu)
            nc.vector.tensor_sub(var, ex2, var)
            sd = rows.tile([1, T], F32, tag="sd", name="sd")
            nc.scalar.activation(sd, var, AF.Sqrt, bias=eps_sb)
            rstd = rows.tile([1, T], F32, tag="rstd", name="rstd")
            nc.vector.reciprocal(rstd, sd)
            c0 = rows.tile([1, T], F32, tag="c0", name="c0")
            nc.vector.tensor_mul(c0, mu, rstd)
            nc.scalar.mul(c0, c0, -1.0)
            RSTD = rows.tile([128, T], F32, tag="RSTD", name="RSTD")
            C0 = rows.tile([128, T], F32, tag="C0", name="C0")
            nc.gpsimd.partition_broadcast(RSTD, rstd)
            nc.gpsimd.partition_broadcast(C0, c0)
            return RSTD, C0

        def ln_apply(pool, dst, src_f32, kt, RSTD, C0, which):
            """dst[:, b*SL:...] = (src*RSTD)*MS + (C0*MS + SH), per batch b."""
            u = pool.tile([128, T], F32, tag="ln_u", name="ln_u")
            nc.vector.tensor_mul(u, src_f32, RSTD)
            for b in range(B):
                sl_ = slice(b * SL, (b + 1) * SL)
                ms = msh_sb[:, kt, b, 2 * which:2 * which + 1]
                sh = msh_sb[:, kt, b, 2 * which + 1:2 * which + 2]
                dd = pool.tile([128, SL], F32, tag="ln_d", name="ln_d")
                nc.vector.tensor_scalar(out=dd, in0=C0[:, sl_], scalar1=ms,
                                        scalar2=sh, op0=MUL, op1=ADD)
                nc.vector.scalar_tensor_tensor(out=dst[:, sl_], in0=u[:, sl_],
                                               scalar=ms, in1=dd, op0=MUL, op1=ADD)

        # ---------- phase 1: LN1 on my tokens -> AllGather xn ----------
        ag_in = dram.tile([NKT, 128, B, SL], F32R)
        ag_out = dram.tile([NC_N, NKT, 128, B, SL], F32R, addr_space="Shared")

        with tc.tile_pool(name="w1p", bufs=1) as w1pool:
            ps_sum = ps_stat.tile([1, T], F32, tag="sum", name="ps_sum")
            ps_ss = ps_stat.tile([1, T], F32, tag="ss", name="ps_ss")
            for kt in range(NKT):
                xr = w1pool.tile([128, T], F32R, tag="xr", bufs=3, name="xr")
                nc.sync.dma_start(out=xr, in_=xfmr[kt])
                nc.tensor.matmul(ps_sum, ones_r, xr, start=(kt == 0), stop=(kt == NKT - 1))
                sq = w1pool.tile([128, T], F32R, tag="sq", bufs=3, name="sq")
                nc.vector.tensor_mul(sq, xr, xr)
                nc.tensor.matmul(ps_ss, ones_r, sq, start=(kt == 0), stop=(kt == NKT - 1))
            RSTD1, C01 = ln_stats_rows(ps_sum, ps_ss)
            for kt in range(NKT):
                x_t = w1pool.tile([128, T], F32, tag="x_t", bufs=2, name="x_t")
                nc.sync.dma_start(out=x_t, in_=xfm[kt])
                xn_t = w1pool.tile([128, T], F32R, tag="xn_t", bufs=3, name="xn_t")
                ln_apply(w1pool, xn_t, x_t, kt, RSTD1, C01, 0)
                nc.sync.dma_start(out=ag_in[kt], in_=xn_t.rearrange("p (b s) -> p b s", b=B))
            nc.gpsimd.collective_compute(
                "AllGather", mybir.AluOpType.bypass,
                replica_groups=[list(range(NC_N))],
                ins=[ag_in[:]], outs=[ag_out[:]])

        a2a_send = dram.tile([NC_N, 128, B, SL], F32R)
        a2a_recv = dram.tile([NC_N, 128, B, SL], F32R)

        with tc.tile_pool(name="attn_res", bufs=1) as ares:
            # ---------- phase 2: QKV for my 2 heads over all tokens ----------
            wq_sb = ares.tile([128, NKT, 128], F32R)
            wk_sb = ares.tile([128, NKT, 128], F32R)
            wv_sb = ares.tile([128, NKT, 128], F32R)
            nc.sync.dma_start(out=wq_sb, in_=wq_t.rearrange("kt p m -> p kt m"))
            nc.sync.dma_start(out=wk_sb, in_=wk_t.rearrange("kt p m -> p kt m"))
            nc.sync.dma_start(out=wv_sb, in_=wv_t.rearrange("kt p m -> p kt m"))
            bqkv_sb = ares.tile([128, 3], F32)
            nc.sync.dma_start(out=bqkv_sb, in_=bqkv)
            qT = ares.tile([128, B, 4, 512], F32R)
            kT = ares.tile([128, B, 4, 512], F32R)
            v_tok = ares.tile([128, B, S // 128, 132], BF16)
            nc.vector.memset(v_tok, 1.0)
            v_tok_hview = v_tok.rearrange("p b t (h x) -> p b t h x", h=2)
            identity_bf = ares.tile([128, 128], BF16)
            from concourse.masks import make_identity
            make_identity(nc, identity_bf)

            with tc.tile_pool(name="w2p", bufs=1) as w2pool, \
                 tc.tile_pool(name="psA", bufs=2, space="PSUM") as psA:
                for b in range(B):
                    for tcc in range(4):
                        psq = psA.tile([128, 512], F32, tag="ps_q", name="psq")
                        psk = psA.tile([128, 512], F32, tag="ps_k", name="psk")
                        psv = psA.tile([128, 512], F32, tag="ps_v", bufs=1, name="psv")
                        for kt in range(NKT):
                            xn_kt = w2pool.tile([128, 512], F32R, tag="xn_kt", bufs=3,
                                                name="xn_kt")
                            for j in range(2):
                                nc.sync.dma_start(
                                    out=xn_kt[:, j * 256:(j + 1) * 256],
                                    in_=ag_out[2 * tcc + j, kt, :, b, :])
                            st, sp = (kt == 0), (kt == NKT - 1)
                            nc.tensor.matmul(psq, wq_sb[:, kt], xn_kt, start=st, stop=sp)
                            nc.tensor.matmul(psk, wk_sb[:, kt], xn_kt, start=st, stop=sp)
                            nc.tensor.matmul(psv, wv_sb[:, kt], xn_kt, start=st, stop=sp)
                        nc.scalar.activation(qT[:, b, tcc], psq, AF.Identity,
                                             bias=bqkv_sb[:, 0:1])
                        nc.scalar.activation(kT[:, b, tcc], psk, AF.Identity,
                                             bias=bqkv_sb[:, 1:2])
                        vstage = w2pool.tile([128, 512], BF16, tag="vstage", bufs=2,
                                             name="vstage")
                        nc.scalar.activation(vstage, psv, AF.Identity,
                                             bias=bqkv_sb[:, 2:3])
                        for j2 in range(4):
                            blk = slice(j2 * 128, (j2 + 1) * 128)
                            pst = psA.tile([128, 128], BF16, tag="ps_t", bufs=1,
                                           name="pst")
                            nc.tensor.transpose(pst, vstage[:, blk], identity_bf)
                            nc.vector.tensor_copy(
                                out=v_tok_hview[:, b, 4 * tcc + j2, :, 0:64],
                                in_=pst.rearrange("p (h e) -> p h e", h=2))

            if dbg:
                nc.sync.dma_start(out=q_dump[:], in_=qT[:])
                nc.sync.dma_start(out=k_dump[:], in_=kT[:])
                with tc.tile_pool(name="dbgp", bufs=1) as dbgp:
                    vf = dbgp.tile([128, B, 16, 132], F32)
                    nc.vector.tensor_copy(out=vf, in_=v_tok)
                    nc.sync.dma_start(out=v_dump[:], in_=vf)

            # ---------- phase 3: attention + AllToAll ----------
            with tc.tile_pool(name="w3p", bufs=1) as w3pool, \
                 tc.tile_pool(name="psB", bufs=1, space="PSUM") as psB:
                for b in range(B):
                    for qc in range(4):
                        ps_o = [psB.tile([65, 512], F32, tag=f"ps_o{h}", name=f"ps_o{h}")
                                for h in range(2)]
                        for kth in range(8):
                            ps_s = [psB.tile([128, 2, 512], F32, tag=f"ps_s{h}",
                                             name=f"ps_s{h}") for h in range(2)]
                            for h in range(2):
                                hp = slice(64 * h, 64 * h + 64)
                                for j in range(2):
                                    kt = 2 * kth + j
                                    nc.tensor.matmul(
                                        ps_s[h][:, j],
                                        kT[hp, b, kt // 4,
                                           (kt % 4) * 128:(kt % 4) * 128 + 128],
                                        qT[hp, b, qc], start=True, stop=True)
                            pt = [w3pool.tile([128, 2, 512], BF16, tag=f"pt{h}", bufs=3,
                                              name=f"pt{h}") for h in range(2)]
                            for h in range(2):
                                nc.scalar.activation(pt[h], ps_s[h], AF.Exp, scale=1.0 / 8.0)
                            if dbg and b == 0 and qc == 0 and kth == 0:
                                ptf2 = w3pool.tile([128, 2, 512], F32, tag="ptf2",
                                                   bufs=1, name="ptf2")
                                nc.vector.tensor_copy(out=ptf2, in_=pt[0])
                                nc.sync.dma_start(out=pt_dump2, in_=ptf2)
                            for h in range(2):
                                off = 66 * h
                                for j in range(2):
                                    kt = 2 * kth + j
                                    nc.tensor.matmul(
                                        ps_o[h], v_tok[:, b, kt, off:off + 65],
                                        pt[h][:, j], start=(kt == 0), stop=(kt == 15))
                        if dbg and b == 0 and qc == 0:
                            pof = w3pool.tile([65, 512], F32, tag="pof", bufs=1,
                                              name="pof")
                            nc.vector.tensor_copy(out=pof, in_=ps_o[0])
                            nc.sync.dma_start(out=po_dump, in_=pof)
                        stage = w3pool.tile([128, 512], F32R, tag="stage", bufs=2,
                                            name="stage")
                        for h in range(2):
                            r = rows.tile([1, 512], F32, tag=f"r{h}", name=f"r{h}")
                            nc.vector.reciprocal(r, ps_o[h][64:65, :])
                            Rb = rows.tile([64, 512], F32, tag=f"Rb{h}", name=f"Rb{h}")
                            nc.gpsimd.partition_broadcast(Rb, r)
                            nc.vector.tensor_mul(stage[64 * h:64 * h + 64],
                                                 ps_o[h][0:64], Rb)
                        for j in range(2):
                            nc.sync.dma_start(out=a2a_send[2 * qc + j, :, b, :],
                                              in_=stage[:, j * 256:(j + 1) * 256])
                if dbg:
                    nc.sync.dma_start(out=send_dump[:], in_=a2a_send[:])
                nc.gpsimd.collective_compute(
                    "AllToAll", mybir.AluOpType.bypass,
                    replica_groups=[list(range(NC_N))],
                    ins=[a2a_send[:]], outs=[a2a_recv[:]])

            # ---------- phase 4: out_proj + residual + LN2 ----------
            wo_b_sb = ares.tile([128, NKT], F32)
            nc.sync.dma_start(out=wo_b_sb, in_=wo_b)
            xa = ares.tile([128, NKT, T], F32R)
            for src in range(NKT):
                nc.sync.dma_start(out=xa[:, src],
                                  in_=a2a_recv[src].rearrange("p b s -> p (b s)"))
            with tc.tile_pool(name="w4p", bufs=1) as w4pool, \
                 tc.tile_pool(name="psC", bufs=2, space="PSUM") as psC:
                ps_sum2 = ps_stat.tile([1, T], F32, tag="sum", name="ps_sum2")
                ps_ss2 = ps_stat.tile([1, T], F32, tag="ss", name="ps_ss2")
                for mt in range(NKT):
                    wo_mt = w4pool.tile([128, NKT, 128], F32R, tag="wo_mt", bufs=2,
                                        name="wo_mt")
                    nc.sync.dma_start(out=wo_mt, in_=wo_t[:, mt].rearrange("s p c -> p s c"))
                    ps = psC.tile([128, T], F32, tag="ps_op", name="ps_op")
                    for src in range(NKT):
                        nc.tensor.matmul(ps, wo_mt[:, src], xa[:, src],
                                         start=(src == 0), stop=(src == NKT - 1))
                    t1 = w4pool.tile([128, T], F32, tag="t1", bufs=3, name="t1")
                    nc.scalar.activation(t1, ps, AF.Identity, bias=wo_b_sb[:, mt:mt + 1])
                    x_t2 = w4pool.tile([128, T], F32, tag="x_t2", bufs=2, name="x_t2")
                    nc.sync.dma_start(out=x_t2, in_=xfm[mt])
                    nc.vector.tensor_add(x2_sb[:, mt], t1, x_t2)
                    x2r = w4pool.tile([128, T], F32R, tag="x2r", bufs=3, name="x2r")
                    nc.vector.tensor_copy(out=x2r, in_=x2_sb[:, mt])
                    nc.tensor.matmul(ps_sum2, ones_r, x2r,
                                     start=(mt == 0), stop=(mt == NKT - 1))
                    sq2 = w4pool.tile([128, T], F32R, tag="sq2", bufs=3, name="sq2")
                    nc.vector.tensor_mul(sq2, x2r, x2r)
                    nc.tensor.matmul(ps_ss2, ones_r, sq2,
                                     start=(mt == 0), stop=(mt == NKT - 1))
                RSTD2, C02 = ln_stats_rows(ps_sum2, ps_ss2)
                for kt in range(NKT):
                    ln_apply(w4pool, xn2[:, kt], x2_sb[:, kt], kt, RSTD2, C02, 1)
                if dbg:
                    nc.sync.dma_start(out=x2_dump[:], in_=x2_sb[:])
                    xaf = w4pool.tile([128, NKT, T], F32, tag="xaf", bufs=1, name="xaf")
                    nc.vector.tensor_copy(out=xaf, in_=xa)
                    nc.sync.dma_start(out=xa_dump[:], in_=xaf)

        # ---------- phase 5: FFN ----------
        with tc.tile_pool(name="ffn_res", bufs=1) as fres, \
             tc.tile_pool(name="w5p", bufs=1) as w5pool, \
             tc.tile_pool(name="psD", bufs=2, space="PSUM") as psD:
            b1_sb = fres.tile([128, NFT], F32)
            nc.sync.dma_start(out=b1_sb, in_=b1_t)
            b2_sb = fres.tile([128, NKT], F32)
            nc.sync.dma_start(out=b2_sb, in_=b2_t)
            h_sb = fres.tile([128, NFT, T], F32R)
            for mtf in range(NFT):
                w1s = w5pool.tile([128, NKT, 128], F32R, tag="w1s", bufs=3, name="w1s")
                nc.sync.dma_start(out=w1s, in_=w1_t[mtf].rearrange("kt p c -> p kt c"))
                ps = psD.tile([128, T], F32, tag="ps_h", name="ps_h")
                for kt in range(NKT):
                    nc.tensor.matmul(ps, w1s[:, kt], xn2[:, kt],
                                     start=(kt == 0), stop=(kt == NKT - 1))
                nc.scalar.activation(h_sb[:, mtf], ps, AF.Gelu, bias=b1_sb[:, mtf:mtf + 1])
            for mto in range(NKT):
                w2s = w5pool.tile([128, NFT, 128], F32R, tag="w2s", bufs=2, name="w2s")
                nc.sync.dma_start(out=w2s, in_=w2_t[mto].rearrange("kt p c -> p kt c"))
                ps = psD.tile([128, T], F32, tag="ps_y", name="ps_y")
                for ktf in range(NFT):
                    nc.tensor.matmul(ps, w2s[:, ktf], h_sb[:, ktf],
                                     start=(ktf == 0), stop=(ktf == NFT - 1))
                t2 = w5pool.tile([128, T], F32, tag="t2", bufs=3, name="t2")
                nc.scalar.activation(t2, ps, AF.Identity, bias=b2_sb[:, mto:mto + 1])
                y_t = w5pool.tile([128, T], F32, tag="y_t", bufs=3, name="y_t")
                nc.vector.tensor_add(y_t, t2, x2_sb[:, mto])
                nc.sync.dma_start(out=yout[mto], in_=y_t.rearrange("p (b s) -> p b s", b=B))

    nc.compile()
    return nc


def _prep_inputs(inputs):
    x = np.ascontiguousarray(inputs["x"], np.float32)
    t_emb = np.asarray(inputs["t_emb"], np.float32)

    t_attn = t_emb @ np.asarray(inputs["wt_attn"], np.float32) + np.asarray(inputs["bt_attn"], np.float32)
    shift_a, scale_a = np.split(t_attn, 2, axis=-1)
    t_ffn = t_emb @ np.asarray(inputs["wt_ffn"], np.float32) + np.asarray(inputs["bt_ffn"], np.float32)
    shift_f, scale_f = np.split(t_ffn, 2, axis=-1)
    ln1_w, ln1_b = np.asarray(inputs["ln1_w"], np.float32), np.asarray(inputs["ln1_b"], np.float32)
    ln2_w, ln2_b = np.asarray(inputs["ln2_w"], np.float32), np.asarray(inputs["ln2_b"], np.float32)
    MS_a = ln1_w[None] * (1.0 + scale_a)          # [B, D]
    SH_a = ln1_b[None] * (1.0 + scale_a) + shift_a
    MS_f = ln2_w[None] * (1.0 + scale_f)
    SH_f = ln2_b[None] * (1.0 + scale_f) + shift_f

    def tile_feat(v):  # [B, D] -> [128, NKT, B]
        return np.transpose(v.reshape(B, NKT, 128), (2, 1, 0))

    msh = np.stack([tile_feat(MS_a), tile_feat(SH_a), tile_feat(MS_f), tile_feat(SH_f)],
                   axis=-1).astype(np.float32)
    msh = np.ascontiguousarray(msh)

    wq = np.asarray(inputs["wq"], np.float32)
    wk = np.asarray(inputs["wk"], np.float32)
    wv = np.asarray(inputs["wv"], np.float32)
    wo = np.asarray(inputs["wo"], np.float32)
    w1 = np.asarray(inputs["w1"], np.float32)
    w2 = np.asarray(inputs["w2"], np.float32)
    bq, bk, bv = (np.asarray(inputs[n], np.float32) for n in ("bq", "bk", "bv"))
    bo, b1, b2 = (np.asarray(inputs[n], np.float32) for n in ("bo", "b1", "b2"))

    wo_tiled = _round_f32r(
        wo.reshape(NKT, 128, NKT, 128).transpose(0, 2, 1, 3))   # [src, mt, r, c]
    w1_tiled = _round_f32r(
        w1.reshape(NKT, 128, NFT, 128).transpose(2, 0, 1, 3))   # [mtf, kt, r, c]
    w2_tiled = _round_f32r(
        w2.reshape(NFT, 128, NKT, 128).transpose(2, 0, 1, 3))   # [mto, ktf, r, c]
    wo_bt = np.ascontiguousarray(bo.reshape(NKT, 128).T)        # [128, mt]
    b1_tl = np.ascontiguousarray(b1.reshape(NFT, 128).T)
    b2_tl = np.ascontiguousarray(b2.reshape(NKT, 128).T)

    in_maps = []
    for c in range(NC_N):
        sl_ = slice(SL * c, SL * (c + 1))
        xc = x[:, sl_, :]                                        # [B, SL, D]
        xfm_c = np.ascontiguousarray(
            xc.transpose(2, 0, 1).reshape(NKT, 128, B, SL))      # [kt, p, b, sl]
        cs = slice(128 * c, 128 * (c + 1))
        in_maps.append({
            "xfm": xfm_c,
            "xfmr": _round_f32r(xfm_c),
            "wq_t": _round_f32r(wq[:, cs].reshape(NKT, 128, 128)),
            "wk_t": _round_f32r(wk[:, cs].reshape(NKT, 128, 128)),
            "wv_t": _round_f32r(wv[:, cs].reshape(NKT, 128, 128)),
            "bqkv": np.ascontiguousarray(np.stack([bq[cs], bk[cs], bv[cs]], axis=1)),
            "wo_t": wo_tiled, "wo_b": wo_bt,
            "w1_t": w1_tiled, "b1_t": b1_tl,
            "w2_t": w2_tiled, "b2_t": b2_tl,
            "msh": msh,
        })
    return in_maps


def kernel(**inputs):
    if "nc" not in _CACHE:
        _CACHE["nc"] = _build()
    nc = _CACHE["nc"]
    in_maps = _prep_inputs(inputs)
    res = run_bass_kernel_spmd(nc, in_maps, core_ids=list(range(NC_N)))
    out = np.empty((B, S, D), np.float32)
    for c in range(NC_N):
        y = res.results[c]["yout"]                               # [kt, p, b, sl]
        out[:, SL * c:SL * (c + 1), :] = y.transpose(2, 3, 0, 1).reshape(B, SL, D)
    return out


# revision 24
# speedup vs baseline: 1.2294x; 1.2294x over previous
"""Trainium2 Bass kernel for a DiT (DiffusionTransformer) block, 8-core SPMD.

Sharding:
  - LayerNorm1+modulate: token-sharded (512 tokens/core), AllGather of xn
    (split in two halves so QKV matmuls overlap the second half).
  - QKV + attention: head-sharded (2 heads x 2 batches per core), all tokens.
  - AllToAll re-shards attention output head-major -> token-major, split per
    batch so A2A(b0) overlaps attention(b1).
  - out_proj + LayerNorm2 + FFN: token-sharded, full weights streamed.

On-chip layout is feature-major ([feature partitions, token free]) everywhere;
matmuls run in float32r (full PE rate, ~tf32 precision); probs/V in bf16.
"""

import sys
from contextlib import ExitStack

sys.path.insert(0, "/opt/trn_rl_repo")

import numpy as np

import concourse.bass as bass
import concourse.tile as tile
from concourse import bacc, mybir
from concourse.bass_utils import run_bass_kernel_spmd
from concourse.masks import make_identity

F32 = mybir.dt.float32
F32R = mybir.dt.float32r
BF16 = mybir.dt.bfloat16
AF = mybir.ActivationFunctionType
MUL = mybir.AluOpType.mult
ADD = mybir.AluOpType.add

NC_N = 8
B, S, D, H, HD, FF = 2, 2048, 1024, 16, 64, 4096
SL = S // NC_N          # 256 seq positions per core (token shard)
T = B * SL              # 512 tokens per core
NKT = D // 128          # 8 feature k-tiles
NFT = FF // 128         # 32 ffn feature tiles
EPS = 1e-5

_CACHE = {}


def _round_f32r(x, bits=10):
    x = np.ascontiguousarray(x, np.float32)
    u = x.view(np.uint32)
    shift = 23 - bits
    u = (u + np.uint32(1 << (shift - 1))) & np.uint32(~((1 << shift) - 1) & 0xFFFFFFFF)
    return u.view(np.float32)


def _build():
    nc = bacc.Bacc("TRN2", target_bir_lowering=False, debug=False, num_devices=NC_N)

    def inp(name, shape, dtype):
        return nc.dram_tensor(name, list(shape), dtype, kind="ExternalInput").ap()

    xfm = inp("xfm", [NKT, 128, B, SL], F32)
    xfmr = inp("xfmr", [NKT, 128, B, SL], F32R)
    wq_t = inp("wq_t", [NKT, 128, 128], F32R)
    wk_t = inp("wk_t", [NKT, 128, 128], F32R)
    wv_t = inp("wv_t", [NKT, 128, 128], F32R)
    bqkv = inp("bqkv", [128, 3], F32)
    wo_t = inp("wo_t", [NKT, NKT, 128, 128], F32R)
    wo_b = inp("wo_b", [128, NKT], F32)
    w1_t = inp("w1_t", [NFT, NKT, 128, 128], F32R)
    b1_t = inp("b1_t", [128, NFT], F32)
    w2_t = inp("w2_t", [NKT, NFT, 128, 128], F32R)
    b2_t = inp("b2_t", [128, NKT], F32)
    msh = inp("msh", [128, NKT, B, 4], F32)
    yout = nc.dram_tensor("yout", [NKT, 128, B, SL], F32, kind="ExternalOutput").ap()

    with tile.TileContext(nc) as tc, ExitStack() as ctx:
        singles = ctx.enter_context(tc.tile_pool(name="singles", bufs=1))
        dram = ctx.enter_context(tc.tile_pool(name="dram", bufs=1, space="DRAM"))
        rows = ctx.enter_context(tc.tile_pool(name="rows", bufs=2))

        # ---------- resident constants ----------
        ones_f = singles.tile([128, 1], F32)
        nc.vector.memset(ones_f, 1.0)
        ones_r = singles.tile([128, 1], F32R)
        nc.vector.tensor_copy(out=ones_r, in_=ones_f)
        eps_sb = singles.tile([1, 1], F32)
        nc.vector.memset(eps_sb, EPS)
        msh_sb = singles.tile([128, NKT, B, 4], F32)
        nc.sync.dma_start(out=msh_sb, in_=msh)
        x2_sb = singles.tile([128, NKT, B, SL], F32)
        xn2 = singles.tile([128, NKT, B, SL], F32R)

        # ---------- helpers ----------
        def ln_stats_rows(ps_sum, ps_ss):
            """[1, T] psum sums -> (RSTD, C0) [128, T] sbuf f32 (broadcasted)."""
            mu = rows.tile([1, T], F32, tag="mu", name="mu")
            ex2 = rows.tile([1, T], F32, tag="ex2", name="ex2")
            nc.vector.tensor_scalar_mul(mu, ps_sum, 1.0 / D)
            nc.vector.tensor_scalar_mul(ex2, ps_ss, 1.0 / D)
            var = rows.tile([1, T], F32, tag="var", name="var")
            nc.vector.tensor_mul(var, mu, mu)
            nc.vector.tensor_sub(var, ex2, var)
            sd = rows.tile([1, T], F32, tag="sd", name="sd")
            nc.scalar.activation(sd, var, AF.Sqrt, bias=eps_sb)
            rstd = rows.tile([1, T], F32, tag="rstd", name="rstd")
            nc.vector.reciprocal(rstd, sd)
            c0 = rows.tile([1, T], F32, tag="c0", name="c0")
            nc.vector.scalar_tensor_tensor(out=c0, in0=mu, scalar=-1.0, in1=rstd,
                                           op0=MUL, op1=MUL)
            RSTD = rows.tile([128, T], F32, tag="RSTD", name="RSTD")
            C0 = rows.tile([128, T], F32, tag="C0", name="C0")
            nc.gpsimd.partition_broadcast(RSTD, rstd)
            nc.gpsimd.partition_broadcast(C0, c0)
            return RSTD, C0

        def ln_apply(pool, dst, src_f32, kt, RSTD, C0, which):
            """dst[:, b, :] = (src*RSTD)*MS + (C0*MS + SH), per batch b.

            dst/src free layout is (b, sl) flattened to T."""
            u = pool.tile([128, T], F32, tag="ln_u", name="ln_u")
            nc.vector.tensor_mul(u, src_f32, RSTD)
            for b in range(B):
                sl_ = slice(b * SL, (b + 1) * SL)
                ms = msh_sb[:, kt, b, 2 * which:2 * which + 1]
                sh = msh_sb[:, kt, b, 2 * which + 1:2 * which + 2]
                dd = pool.tile([128, SL], F32, tag="ln_d", name="ln_d")
                nc.vector.tensor_scalar(out=dd, in0=C0[:, sl_], scalar1=ms,
                                        scalar2=sh, op0=MUL, op1=ADD)
                nc.vector.scalar_tensor_tensor(out=dst[:, sl_], in0=u[:, sl_],
                                               scalar=ms, in1=dd, op0=MUL, op1=ADD)

        # ---------- phase 1: LN1 on my tokens -> AllGather xn (2 halves) ----------
        ag_in = [dram.tile([4, 128, B, SL], F32R, name=f"ag_in{i}") for i in range(2)]
        ag_out = [dram.tile([NC_N, 4, 128, B, SL], F32R, addr_space="Shared",
                            name=f"ag_out{i}") for i in range(2)]

        with tc.tile_pool(name="w1p", bufs=1) as w1pool, \
             tc.tile_pool(name="psS1", bufs=1, space="PSUM") as psS1:
            ps_sum = psS1.tile([1, T], F32, tag="sum", name="ps_sum")
            ps_ss = psS1.tile([1, T], F32, tag="ss", name="ps_ss")
            for kt in range(NKT):
                xr = w1pool.tile([128, T], F32R, tag="xr", bufs=3, name="xr")
                nc.sync.dma_start(out=xr, in_=xfmr[kt])
                nc.tensor.matmul(ps_sum, ones_r, xr, start=(kt == 0), stop=(kt == NKT - 1))
                sq = w1pool.tile([128, T], F32R, tag="sq", bufs=3, name="sq")
                nc.vector.tensor_mul(sq, xr, xr)
                nc.tensor.matmul(ps_ss, ones_r, sq, start=(kt == 0), stop=(kt == NKT - 1))
            RSTD1, C01 = ln_stats_rows(ps_sum, ps_ss)
            for kt in range(NKT):
                x_t = w1pool.tile([128, T], F32, tag="x_t", bufs=2, name="x_t")
                nc.sync.dma_start(out=x_t, in_=xfm[kt])
                xn_t = w1pool.tile([128, T], F32R, tag="xn_t", bufs=3, name="xn_t")
                ln_apply(w1pool, xn_t, x_t, kt, RSTD1, C01, 0)
                nc.sync.dma_start(out=ag_in[kt // 4][kt % 4],
                                  in_=xn_t.rearrange("p (b s) -> p b s", b=B))
                if kt % 4 == 3:
                    nc.gpsimd.collective_compute(
                        "AllGather", mybir.AluOpType.bypass,
                        replica_groups=[list(range(NC_N))],
                        ins=[ag_in[kt // 4][:]], outs=[ag_out[kt // 4][:]])

        a2a_send = [dram.tile([NC_N, 128, SL], F32R, name=f"a2a_send{b}") for b in range(B)]
        a2a_recv = [dram.tile([NC_N, 128, SL], F32R, name=f"a2a_recv{b}") for b in range(B)]

        with tc.tile_pool(name="attn_res", bufs=1) as ares:
            # ---------- phase 2: QKV for my 2 heads over all tokens ----------
            wq_sb = ares.tile([128, NKT, 128], F32R)
            wk_sb = ares.tile([128, NKT, 128], F32R)
            wv_sb = ares.tile([128, NKT, 128], F32R)
            nc.sync.dma_start(out=wq_sb, in_=wq_t.rearrange("kt p m -> p kt m"))
            nc.sync.dma_start(out=wk_sb, in_=wk_t.rearrange("kt p m -> p kt m"))
            nc.sync.dma_start(out=wv_sb, in_=wv_t.rearrange("kt p m -> p kt m"))
            bqkv_sb = ares.tile([128, 3], F32)
            nc.sync.dma_start(out=bqkv_sb, in_=bqkv)
            qT = ares.tile([128, B, 4, 512], F32R)
            kT = ares.tile([128, B, 4, 512], F32R)
            v_tok = ares.tile([128, B, S // 128, 132], BF16)
            nc.vector.memset(v_tok, 1.0)
            v_tok_hview = v_tok.rearrange("p b t (h x) -> p b t h x", h=2)
            identity_bf = ares.tile([128, 128], BF16)
            make_identity(nc, identity_bf)

            with tc.tile_pool(name="w2p", bufs=1) as w2pool, \
                 tc.tile_pool(name="psA", bufs=1, space="PSUM") as psA:
                for tcc in range(4):
                    psq = psA.tile([128, B, 512], F32, tag="ps_q", name="psq")
                    psk = psA.tile([128, B, 512], F32, tag="ps_k", name="psk")
                    psv = psA.tile([128, B, 512], F32, tag="ps_v", name="psv")
                    for kt in range(NKT):
                        xn_kt = w2pool.tile([128, 2, B, SL], F32R, tag="xn_kt", bufs=3,
                                            name="xn_kt")
                        nc.sync.dma_start(
                            out=xn_kt,
                            in_=ag_out[kt // 4][2 * tcc:2 * tcc + 2, kt % 4].rearrange(
                                "j p b s -> p j b s"))
                        st, sp = (kt == 0), (kt == NKT - 1)
                        for b in range(B):
                            rhs = xn_kt[:, :, b, :]
                            nc.tensor.matmul(psq[:, b], wq_sb[:, kt], rhs, start=st, stop=sp)
                            nc.tensor.matmul(psk[:, b], wk_sb[:, kt], rhs, start=st, stop=sp)
                            nc.tensor.matmul(psv[:, b], wv_sb[:, kt], rhs, start=st, stop=sp)
                    for b in range(B):
                        nc.vector.tensor_scalar(out=qT[:, b, tcc], in0=psq[:, b],
                                                scalar1=bqkv_sb[:, 0:1], scalar2=None,
                                                op0=ADD)
                        nc.vector.tensor_scalar(out=kT[:, b, tcc], in0=psk[:, b],
                                                scalar1=bqkv_sb[:, 1:2], scalar2=None,
                                                op0=ADD)
                        vstage = w2pool.tile([128, 512], BF16, tag="vstage", bufs=2,
                                             name="vstage")
                        nc.scalar.activation(vstage, psv[:, b], AF.Identity,
                                             bias=bqkv_sb[:, 2:3])
                        for j2 in range(4):
                            blk = slice(j2 * 128, (j2 + 1) * 128)
                            pst = psA.tile([128, 128], BF16, tag="ps_t", bufs=2,
                                           name="pst")
                            nc.tensor.transpose(pst, vstage[:, blk], identity_bf)
                            nc.vector.tensor_copy(
                                out=v_tok_hview[:, b, 4 * tcc + j2, :, 0:64],
                                in_=pst.rearrange("p (h e) -> p h e", h=2))

            # ---------- phase 3: attention; per-batch AllToAll ----------
            with tc.tile_pool(name="w3p", bufs=1) as w3pool, \
                 tc.tile_pool(name="psB", bufs=1, space="PSUM") as psB:
                for b in range(B):
                    for qc in range(4):
                        ps_o = [psB.tile([65, 512], F32, tag=f"ps_o{h}", bufs=2,
                                         name=f"ps_o{h}") for h in range(2)]
                        for kt in range(16):
                            ps_s = psB.tile([128, 2, 512], F32, tag="ps_s", bufs=2,
                                            name="ps_s")
                            for h in range(2):
                                hp = slice(64 * h, 64 * h + 64)
                                nc.tensor.matmul(
                                    ps_s[:, h],
                                    kT[hp, b, kt // 4,
                                       (kt % 4) * 128:(kt % 4) * 128 + 128],
                                    qT[hp, b, qc], start=True, stop=True)
                            pt = w3pool.tile([128, 2, 512], BF16, tag="pt", bufs=3,
                                             name="pt")
                            nc.scalar.activation(pt, ps_s, AF.Exp, scale=1.0 / 8.0)
                            for h in range(2):
                                off = 66 * h
                                nc.tensor.matmul(
                                    ps_o[h], v_tok[:, b, kt, off:off + 65],
                                    pt[:, h], start=(kt == 0), stop=(kt == 15))
                        stage = w3pool.tile([128, 512], F32R, tag="stage", bufs=2,
                                            name="stage")
                        for h in range(2):
                            r = rows.tile([1, 512], F32, tag=f"r{h}", name=f"r{h}")
                            nc.vector.reciprocal(r, ps_o[h][64:65, :])
                            Rb = rows.tile([64, 512], F32, tag=f"Rb{h}", name=f"Rb{h}")
                            nc.gpsimd.partition_broadcast(Rb, r)
                            nc.vector.tensor_mul(stage[64 * h:64 * h + 64],
                                                 ps_o[h][0:64], Rb)
                        for j in range(2):
                            nc.sync.dma_start(out=a2a_send[b][2 * qc + j],
                                              in_=stage[:, j * 256:(j + 1) * 256])
                    nc.gpsimd.collective_compute(
                        "AllToAll", mybir.AluOpType.bypass,
                        replica_groups=[list(range(NC_N))],
                        ins=[a2a_send[b][:]], outs=[a2a_recv[b][:]])

            # ---------- phase 4: out_proj + residual + LN2 ----------
            wo_b_sb = ares.tile([128, NKT], F32)
            nc.sync.dma_start(out=wo_b_sb, in_=wo_b)
            xa = ares.tile([128, NKT, B, SL], F32R)
            for b in range(B):
                for src in range(NKT):
                    nc.sync.dma_start(out=xa[:, src, b], in_=a2a_recv[b][src])
            with tc.tile_pool(name="w4p", bufs=1) as w4pool, \
                 tc.tile_pool(name="psC", bufs=1, space="PSUM") as psC:
                ps_sum2 = psC.tile([1, T], F32, tag="sum2", name="ps_sum2")
                ps_ss2 = psC.tile([1, T], F32, tag="ss2", name="ps_ss2")
                for mt in range(NKT):
                    wo_mt = w4pool.tile([128, NKT, 128], F32R, tag="wo_mt", bufs=2,
                                        name="wo_mt")
                    nc.sync.dma_start(out=wo_mt, in_=wo_t[:, mt].rearrange("s p c -> p s c"))
                    x_t2 = w4pool.tile([128, B, SL], F32, tag="x_t2", bufs=2, name="x_t2")
                    nc.sync.dma_start(out=x_t2, in_=xfm[mt])
                    ps = psC.tile([128, B, SL], F32, tag="ps_op", bufs=2, name="ps_op")
                    for b in range(B):
                        for src in range(NKT):
                            nc.tensor.matmul(ps[:, b], wo_mt[:, src], xa[:, src, b],
                                             start=(src == 0), stop=(src == NKT - 1))
                    nc.vector.scalar_tensor_tensor(
                        out=x2_sb[:, mt], in0=ps.rearrange("p b s -> p (b s)"),
                        scalar=wo_b_sb[:, mt:mt + 1],
                        in1=x_t2.rearrange("p b s -> p (b s)"), op0=ADD, op1=ADD)
                    x2r = w4pool.tile([128, T], F32R, tag="x2r", bufs=3, name="x2r")
                    nc.vector.tensor_copy(out=x2r, in_=x2_sb[:, mt].rearrange("p b s -> p (b s)"))
                    nc.tensor.matmul(ps_sum2, ones_r, x2r,
                                     start=(mt == 0), stop=(mt == NKT - 1))
                    sq2 = w4pool.tile([128, T], F32R, tag="sq2", bufs=3, name="sq2")
                    nc.vector.tensor_mul(sq2, x2r, x2r)
                    nc.tensor.matmul(ps_ss2, ones_r, sq2,
                                     start=(mt == 0), stop=(mt == NKT - 1))
                RSTD2, C02 = ln_stats_rows(ps_sum2, ps_ss2)
                for kt in range(NKT):
                    ln_apply(w4pool, xn2[:, kt].rearrange("p b s -> p (b s)"),
                             x2_sb[:, kt].rearrange("p b s -> p (b s)"),
                             kt, RSTD2, C02, 1)

        # ---------- phase 5: FFN ----------
        with tc.tile_pool(name="ffn_res", bufs=1) as fres, \
             tc.tile_pool(name="w5p", bufs=1) as w5pool, \
             tc.tile_pool(name="psD", bufs=2, space="PSUM") as psD:
            b1_sb = fres.tile([128, NFT], F32)
            nc.sync.dma_start(out=b1_sb, in_=b1_t)
            b2_sb = fres.tile([128, NKT], F32)
            nc.sync.dma_start(out=b2_sb, in_=b2_t)
            h_sb = fres.tile([128, NFT, T], F32R)
            xn2f = xn2.rearrange("p kt b s -> p kt (b s)")
            for mtf in range(NFT):
                w1s = w5pool.tile([128, NKT, 128], F32R, tag="w1s", bufs=3, name="w1s")
                nc.sync.dma_start(out=w1s, in_=w1_t[mtf].rearrange("kt p c -> p kt c"))
                ps = psD.tile([128, T], F32, tag="ps_h", name="ps_h")
                for kt in range(NKT):
                    nc.tensor.matmul(ps, w1s[:, kt], xn2f[:, kt],
                                     start=(kt == 0), stop=(kt == NKT - 1))
                nc.scalar.activation(h_sb[:, mtf], ps, AF.Gelu, bias=b1_sb[:, mtf:mtf + 1])
            for mto in range(NKT):
                w2s = w5pool.tile([128, NFT, 128], F32R, tag="w2s", bufs=2, name="w2s")
                nc.sync.dma_start(out=w2s, in_=w2_t[mto].rearrange("kt p c -> p kt c"))
                ps = psD.tile([128, T], F32, tag="ps_y", name="ps_y")
                for ktf in range(NFT):
                    nc.tensor.matmul(ps, w2s[:, ktf], h_sb[:, ktf],
                                     start=(ktf == 0), stop=(ktf == NFT - 1))
                y_t = w5pool.tile([128, T], F32, tag="y_t", bufs=3, name="y_t")
                nc.vector.scalar_tensor_tensor(
                    out=y_t, in0=ps, scalar=b2_sb[:, mto:mto + 1],
                    in1=x2_sb[:, mto].rearrange("p b s -> p (b s)"), op0=ADD, op1=ADD)
                nc.sync.dma_start(out=yout[mto], in_=y_t.rearrange("p (b s) -> p b s", b=B))

    nc.compile()
    return nc


def _prep_inputs(inputs):
    x = np.ascontiguousarray(inputs["x"], np.float32)
    t_emb = np.asarray(inputs["t_emb"], np.float32)

    t_attn = t_emb @ np.asarray(inputs["wt_attn"], np.float32) + np.asarray(inputs["bt_attn"], np.float32)
    shift_a, scale_a = np.split(t_attn, 2, axis=-1)
    t_ffn = t_emb @ np.asarray(inputs["wt_ffn"], np.float32) + np.asarray(inputs["bt_ffn"], np.float32)
    shift_f, scale_f = np.split(t_ffn, 2, axis=-1)
    ln1_w, ln1_b = np.asarray(inputs["ln1_w"], np.float32), np.asarray(inputs["ln1_b"], np.float32)
    ln2_w, ln2_b = np.asarray(inputs["ln2_w"], np.float32), np.asarray(inputs["ln2_b"], np.float32)
    MS_a = ln1_w[None] * (1.0 + scale_a)          # [B, D]
    SH_a = ln1_b[None] * (1.0 + scale_a) + shift_a
    MS_f = ln2_w[None] * (1.0 + scale_f)
    SH_f = ln2_b[None] * (1.0 + scale_f) + shift_f

    def tile_feat(v):  # [B, D] -> [128, NKT, B]
        return np.transpose(v.reshape(B, NKT, 128), (2, 1, 0))

    msh = np.stack([tile_feat(MS_a), tile_feat(SH_a), tile_feat(MS_f), tile_feat(SH_f)],
                   axis=-1).astype(np.float32)
    msh = np.ascontiguousarray(msh)

    wq = np.asarray(inputs["wq"], np.float32)
    wk = np.asarray(inputs["wk"], np.float32)
    wv = np.asarray(inputs["wv"], np.float32)
    wo = np.asarray(inputs["wo"], np.float32)
    w1 = np.asarray(inputs["w1"], np.float32)
    w2 = np.asarray(inputs["w2"], np.float32)
    bq, bk, bv = (np.asarray(inputs[n], np.float32) for n in ("bq", "bk", "bv"))
    bo, b1, b2 = (np.asarray(inputs[n], np.float32) for n in ("bo", "b1", "b2"))

    wo_tiled = _round_f32r(
        wo.reshape(NKT, 128, NKT, 128).transpose(0, 2, 1, 3))   # [src, mt, r, c]
    w1_tiled = _round_f32r(
        w1.reshape(NKT, 128, NFT, 128).transpose(2, 0, 1, 3))   # [mtf, kt, r, c]
    w2_tiled = _round_f32r(
        w2.reshape(NFT, 128, NKT, 128).transpose(2, 0, 1, 3))   # [mto, ktf, r, c]
    wo_bt = np.ascontiguousarray(bo.reshape(NKT, 128).T)        # [128, mt]
    b1_tl = np.ascontiguousarray(b1.reshape(NFT, 128).T)
    b2_tl = np.ascontiguousarray(b2.reshape(NKT, 128).T)

    in_maps = []
    for c in range(NC_N):
        sl_ = slice(SL * c, SL * (c + 1))
        xc = x[:, sl_, :]                                        # [B, SL, D]
        xfm_c = np.ascontiguousarray(
            xc.transpose(2, 0, 1).reshape(NKT, 128, B, SL))      # [kt, p, b, sl]
        cs = slice(128 * c, 128 * (c + 1))
        in_maps.append({
            "xfm": xfm_c,
            "xfmr": _round_f32r(xfm_c),
            "wq_t": _round_f32r(wq[:, cs].reshape(NKT, 128, 128)),
            "wk_t": _round_f32r(wk[:, cs].reshape(NKT, 128, 128)),
            "wv_t": _round_f32r(wv[:, cs].reshape(NKT, 128, 128)),
            "bqkv": np.ascontiguousarray(np.stack([bq[cs], bk[cs], bv[cs]], axis=1)),
            "wo_t": wo_tiled, "wo_b": wo_bt,
            "w1_t": w1_tiled, "b1_t": b1_tl,
            "w2_t": w2_tiled, "b2_t": b2_tl,
            "msh": msh,
        })
    return in_maps


def kernel(**inputs):
    if "nc" not in _CACHE:
        _CACHE["nc"] = _build()
    nc = _CACHE["nc"]
    in_maps = _prep_inputs(inputs)
    res = run_bass_kernel_spmd(nc, in_maps, core_ids=list(range(NC_N)))
    out = np.empty((B, S, D), np.float32)
    for c in range(NC_N):
        y = res.results[c]["yout"]                               # [kt, p, b, sl]
        out[:, SL * c:SL * (c + 1), :] = y.transpose(2, 3, 0, 1).reshape(B, SL, D)
    return out
